# revision 9
# baseline (speedup 1.0000x reference)
"""Trainium2 Bass kernel for CausalSelfAttention (GQA + RoPE + sliding window).

v4: bf16 data path (fp32 PSUM accumulation + fp32 RoPE + fp32 output),
per-(qc,h) softmax denominator via one ones-matmul over a DVE-accumulated
exp sum (drops the per-k-tile den matmuls from PE), out-projection
pieces interleaved into the NEXT q-chunk's attention so PE fills the
ACT exp-wait slots, and f-outer/e-inner projection chains for chunks
s1..s3 so each feature's RoPE eviction pipelines under the next
feature's matmuls (s0 stays e-outer for DMA-fill friendliness).

Module: B=2, S=2048, E=2048, NH=16 heads, NKV=4 kv heads, HD=128,
WINDOW=1024 (local causal: 0 <= q-k < 1024), fp32 in/out.

Sharding (8 cores): core = b*4 + g  where b = batch (2), g = kv-head group
(4). Each core handles 1 batch x 1 kv head (4 q heads); host sums the 4
partial out-projections per batch.

Layouts: xT [E,S] bf16, qT/kT [128,S] bf16 (RoPE computed in fp32 during
PSUM eviction, stored bf16), v natural [S,128] bf16 via PE transposes,
scoresT [k,q] fp32 PSUM -> exp -> ex bf16, exp row-sum accumulated in
bf16 SBUF (exs), den = ones^T @ exs, yT [E,S] fp32 partial output.
"""

import math
import os

import numpy as np

B, S, E = 2, 2048, 2048
NH, NKV, HD = 16, 4, 128
WINDOW = 1024
P = 128
QC = 512  # q chunk (moving free dim)
HC = 256  # half chunk for partial tiles
N_QC = S // QC  # 4
N_E = E // P  # 16 contraction chunks
SCALE = 1.0 / math.sqrt(HD)

# mask deltas: delta = q0 - 128*kt for partially-masked [k=128, q] tiles.
MASK_DELTAS = [-384, -256, -128, 0, 640, 768, 896, 1024]
MASK_IDX = {d: i for i, d in enumerate(MASK_DELTAS)}


def _kt_range(qc):
    kt_lo = max(0, (qc * QC - (WINDOW - 1)) // P)
    kt_hi = (qc * QC + QC - 1) // P
    return list(range(kt_lo, kt_hi + 1))


def _full_partial(qc):
    full, units = [], []
    for kt in _kt_range(qc):
        d = QC * qc - P * kt
        if 128 <= d <= 512:
            full.append(kt)
            continue
        for h2 in range(2):
            dh = d + h2 * HC
            lo, hi = dh - (P - 1), dh + (HC - 1)
            if hi < 0 or lo >= WINDOW:
                continue  # fully masked
            if lo >= 0 and hi < WINDOW:
                units.append((kt, h2, None))  # fully valid
            else:
                assert dh in MASK_IDX, (qc, kt, h2, dh)
                units.append((kt, h2, dh))
    return full, units


def _ops_for(qc):
    full_kts, units = _full_partial(qc)
    ops = []
    for i in range(0, len(full_kts), 2):
        ops.append(("full_pair", full_kts[i:i + 2]))
    for i in range(0, len(units), 2):
        ops.append(("unit_pair", units[i:i + 2]))
    return ops


def build_nc():
    import concourse.bass as bass
    import concourse.mybir as mybir
    import concourse.tile as tile
    from concourse import bacc
    from concourse.masks import make_identity

    f32 = mybir.dt.float32
    f32r = mybir.dt.float32r
    bf = mybir.dt.bfloat16
    Exp = mybir.ActivationFunctionType.Exp

    nc = bacc.Bacc("TRN2", target_bir_lowering=False, debug=False, num_devices=8)

    xT = nc.dram_tensor("xT", [E, S], bf, kind="ExternalInput")
    wqkvT = nc.dram_tensor("wqkvT", [E, 768], bf, kind="ExternalInput")
    woT = nc.dram_tensor("woT", [4 * P, E], bf, kind="ExternalInput")
    cosT = nc.dram_tensor("cosT", [P, S], f32, kind="ExternalInput")
    sinFT = nc.dram_tensor("sinFT", [P, S], f32, kind="ExternalInput")
    masks = nc.dram_tensor(
        "masks", [len(MASK_DELTAS), P, QC], bf, kind="ExternalInput"
    )
    y = nc.dram_tensor("y", [E, S], f32, kind="ExternalOutput")  # yT layout

    loop = int(os.environ.get("BASS_BENCH_LOOP", "1"))

    with tile.TileContext(nc) as tc:
        with (
            tc.tile_pool(name="persist", bufs=1) as pp,
            tc.tile_pool(name="wo_pool", bufs=1) as wop,
        ):
          for it in range(loop):
            I = f"i{it}_"
            qT_r = [pp.tile([P, S], bf, tag=f"qT{h}", name=f"{I}qT{h}") for h in range(4)]
            kT_r = pp.tile([P, S], bf, tag="kT", name=I + "kT")
            v_nat = pp.tile([P, S], bf, tag="v_nat", name=I + "v_nat")
            ident = pp.tile([P, P], f32, tag="ident", name=I + "ident")
            make_identity(nc, ident[:])
            ident_bf = pp.tile([P, P], bf, tag="ident_bf", name=I + "ident_bf")
            nc.vector.tensor_copy(ident_bf[:], ident[:])
            ones_col_f = pp.tile([P, 1], f32, tag="ones_col_f", name=I + "ones_col_f")
            ones_col = pp.tile([P, 1], bf, tag="ones_col", name=I + "ones_col")
            nc.vector.memset(ones_col_f[:], 1.0)
            nc.vector.tensor_copy(ones_col[:], ones_col_f[:])

            # ---------------- Phase 1: QKV projections + RoPE + v transpose
            with (
                tc.tile_pool(name=I + "wqkv_pool", bufs=1) as wqp,
                tc.tile_pool(name=I + "xpool", bufs=12) as xp,
                tc.tile_pool(name=I + "cspool", bufs=2) as csp,
                tc.tile_pool(name=I + "vstage", bufs=2) as vsp,
                tc.tile_pool(name=I + "proj_ps", bufs=1, space="PSUM") as pps,
                tc.tile_pool(name=I + "vtr_ps", bufs=1, space="PSUM") as vtps,
            ):
                # paired DMAs: 2 e-blocks per transfer
                wqkv2 = []
                x_pre = {}
                for ep2 in range(N_E // 2):
                    t = wqp.tile([P, 2 * 768], bf, tag=f"wqkv{ep2}", name=f"{I}wqkv{ep2}")
                    nc.sync.dma_start(
                        out=t[:].rearrange("p (b f) -> p b f", b=2),
                        in_=wqkvT[ep2 * 2 * P:(ep2 + 1) * 2 * P, :].rearrange(
                            "(b p) f -> p b f", b=2
                        ),
                    )
                    wqkv2.append(t)
                    x2 = xp.tile(
                        [P, 2 * QC], bf, tag="x_r", bufs=12, name=f"{I}x_r0_{ep2}"
                    )
                    nc.scalar.dma_start(
                        out=x2[:].rearrange("p (b q) -> p b q", b=2),
                        in_=xT[ep2 * 2 * P:(ep2 + 1) * 2 * P, 0:QC].rearrange(
                            "(b p) q -> p b q", b=2
                        ),
                    )
                    x_pre[(0, ep2)] = x2
                wqkv_sl = [
                    wqkv2[e // 2][:, (e % 2) * 768:(e % 2 + 1) * 768]
                    for e in range(N_E)
                ]

                cos_all = csp.tile([P, S], f32, tag="cos_all", bufs=1, name=I + "cos_all")
                sinF_all = csp.tile([P, S], f32, tag="sinF_all", bufs=1, name=I + "sinF_all")
                nc.scalar.dma_start(out=cos_all[:], in_=cosT[:])
                nc.scalar.dma_start(out=sinF_all[:], in_=sinFT[:])

                for s in range(N_QC):
                    ssl = slice(s * QC, (s + 1) * QC)
                    cos_sb = cos_all[:, ssl]
                    sinF_sb = sinF_all[:, ssl]

                    ps = [
                        pps.tile(
                            [P, QC], f32,
                            tag=f"proj{(f + s) % 7}",
                            name=f"{I}proj{f}_{s}",
                        )
                        for f in range(6)
                    ]
                    x_sl = {}
                    for ep2 in range(N_E // 2):
                        if (s, ep2) in x_pre:
                            x2 = x_pre[(s, ep2)]
                        else:
                            x2 = xp.tile(
                                [P, 2 * QC], bf, tag="x_r", bufs=12,
                                name=f"{I}x_r{s}_{ep2}",
                            )
                            nc.scalar.dma_start(
                                out=x2[:].rearrange("p (b q) -> p b q", b=2),
                                in_=xT[ep2 * 2 * P:(ep2 + 1) * 2 * P, ssl].rearrange(
                                    "(b p) q -> p b q", b=2
                                ),
                            )
                        x_sl[2 * ep2] = x2[:, 0:QC]
                        x_sl[2 * ep2 + 1] = x2[:, QC:2 * QC]

                    # s0: e-outer (each step needs only tile e -- friendly
                    # to the initial DMA fill). s1+: f-outer/e-inner so each
                    # feature's PSUM finishes early and its RoPE eviction
                    # pipelines under the next feature's matmul chain.
                    def fchain(f):
                        if s == 0 and f == 1:
                            # emit the whole chunk e-outer on the first call;
                            # later fchain calls for s0 are no-ops
                            for e in range(N_E):
                                for f_ in range(6):
                                    nc.tensor.matmul(
                                        ps[f_][:],
                                        wqkv_sl[e][:, f_ * P:(f_ + 1) * P],
                                        x_sl[e],
                                        start=(e == 0),
                                        stop=(e == N_E - 1),
                                    )
                        elif s == 0:
                            return
                        else:
                            for e in range(N_E):
                                nc.tensor.matmul(
                                    ps[f][:],
                                    wqkv_sl[e][:, f * P:(f + 1) * P],
                                    x_sl[e],
                                    start=(e == 0),
                                    stop=(e == N_E - 1),
                                )

                    # RoPE on fp32, stored bf16. Evict in the order the next
                    # chunk needs PSUM slots: q1,q2,q3,k,v,q0.
                    def rope_evict(dst, psrc, tmp_name):
                        stage = xp.tile(
                            [P, QC], f32, tag="rstage", bufs=3, name="st" + tmp_name
                        )
                        nc.scalar.copy(stage[:], psrc)
                        shf = xp.tile([P, QC], f32, tag="rope_shf", name="sh" + tmp_name)
                        H = P // 2
                        nc.vector.tensor_copy(shf[0:H, :], stage[H:P, :])
                        nc.vector.tensor_copy(shf[H:P, :], stage[0:H, :])
                        nc.vector.tensor_mul(shf[:], shf[:], sinF_sb)
                        nc.vector.tensor_mul(stage[:], stage[:], cos_sb)
                        nc.vector.tensor_add(dst, stage[:], shf[:])

                    for h in (1, 2, 3):
                        fchain(h)
                        rope_evict(qT_r[h][:, ssl], ps[h][:], f"{I}rope_q{h}_{s}")
                    fchain(4)
                    rope_evict(kT_r[:, ssl], ps[4][:], f"{I}rope_k{s}")
                    fchain(5)
                    v_sb = vsp.tile([P, QC], bf, tag="v_sb", name=f"{I}v_sb{s}")
                    nc.scalar.copy(v_sb[:], ps[5][:])
                    fchain(0)
                    rope_evict(qT_r[0][:, ssl], ps[0][:], f"{I}rope_q0_{s}")
                    for j in range(QC // P):
                        kt = s * (QC // P) + j
                        tps = vtps.tile([P, P], bf, tag="vtr", name=f"{I}vtr{kt}")
                        nc.tensor.transpose(
                            tps[:], v_sb[:, j * P:(j + 1) * P], ident_bf[:]
                        )
                        nc.scalar.copy(v_nat[:, kt * P:(kt + 1) * P], tps[:])

            # Wo resident load
            wo_r = []
            for d in range(4):
                t = wop.tile([P, E], bf, tag=f"wo_r{d}", name=f"{I}wo_r{d}")
                nc.sync.dma_start(out=t[:], in_=woT[d * P:(d + 1) * P, :])
                wo_r.append(t)

            # ---------------- Phase 2+3: attention with interleaved out-proj
            with (
                tc.tile_pool(name=I + "mask_pool", bufs=1) as mp,
                tc.tile_pool(name=I + "exp_pool", bufs=6) as ep,
                tc.tile_pool(name=I + "exs_pool", bufs=2) as esp,
                tc.tile_pool(name=I + "outT_pool", bufs=1) as op_,
                tc.tile_pool(name=I + "small_pool", bufs=3) as sp,
                tc.tile_pool(name=I + "sc_ps", bufs=2, space="PSUM") as scp,
                tc.tile_pool(name=I + "pv_ps", bufs=2, space="PSUM") as pvp,
                tc.tile_pool(name=I + "den_ps", bufs=1, space="PSUM") as dbp,
                tc.tile_pool(name=I + "op_ps", bufs=1, space="PSUM") as opp,
            ):
                nmask = len(MASK_DELTAS)
                mask_all = mp.tile(
                    [P, nmask * QC], bf, tag="mask_all", name=I + "mask_all"
                )
                nc.sync.dma_start(
                    out=mask_all[:].rearrange("p (m q) -> p m q", m=nmask),
                    in_=masks[:].rearrange("m p q -> p m q"),
                )
                mask_sb = [
                    mask_all[:, m * QC:(m + 1) * QC] for m in range(nmask)
                ]

                outT = [
                    op_.tile([P, S], bf, tag=f"outT{h}", name=f"{I}outT{h}")
                    for h in range(4)
                ]

                def emit_qk(qc, h, oi, op):
                    """QK matmuls + exp (+ mask muls) for one op; returns ex."""
                    kind, pl = op
                    qsl_ = slice(qc * QC, (qc + 1) * QC)
                    w = QC if kind == "full_pair" else HC
                    sc = scp.tile(
                        [P, 2 * QC], f32, tag="sc", name=f"{I}sc{qc}_{h}_{oi}"
                    )
                    for j, item in enumerate(pl):
                        if kind == "full_pair":
                            kt = item
                            qs = qsl_
                        else:
                            kt, h2, dh = item
                            q0 = qc * QC + h2 * HC
                            qs = slice(q0, q0 + HC)
                        nc.tensor.matmul(
                            sc[:, j * w:(j + 1) * w],
                            kT_r[:, kt * P:(kt + 1) * P],
                            qT_r[h][:, qs],
                            start=True,
                            stop=True,
                        )
                    ex = ep.tile(
                        [P, 2 * QC], bf, tag="ex", name=f"{I}ex{qc}_{h}_{oi}"
                    )
                    nc.scalar.activation(
                        ex[:, : len(pl) * w],
                        sc[:, : len(pl) * w],
                        Exp,
                        scale=SCALE,
                    )
                    if kind == "unit_pair":
                        for j, (kt, h2, dh) in enumerate(pl):
                            if dh is not None:
                                nc.vector.tensor_mul(
                                    ex[:, j * w:(j + 1) * w],
                                    ex[:, j * w:(j + 1) * w],
                                    mask_sb[MASK_IDX[dh]][:, :HC],
                                )
                    return ex

                def emit_op_piece(qc_prev, e, idx):
                    """One out-projection e-block for q chunk qc_prev."""
                    qsl_p = slice(qc_prev * QC, (qc_prev + 1) * QC)
                    yp = opp.tile([P, QC], f32, tag="yp", name=f"{I}yp{qc_prev}_{e}")
                    for d in range(4):
                        nc.tensor.matmul(
                            yp[:],
                            wo_r[d][:, e * P:(e + 1) * P],
                            outT[d][:, qsl_p],
                            start=(d == 0),
                            stop=(d == 3),
                        )
                    y2 = sp.tile([P, QC], f32, tag="y_sb", name=f"{I}ysb{qc_prev}_{e}")
                    if idx % 2 == 0:
                        nc.scalar.copy(y2[:], yp[:])
                    else:
                        nc.vector.tensor_copy(y2[:], yp[:])
                    nc.sync.dma_start(out=y[e * P:(e + 1) * P, qsl_p], in_=y2[:])

                pending = {}
                for qc in range(N_QC):
                    qsl = slice(qc * QC, (qc + 1) * QC)
                    ops = _ops_for(qc)
                    n_acc = sum(len(pl) for _, pl in ops)
                    n_ops_qc = 4 * len(ops)
                    pieces_done = 0
                    g = 0  # global op index within this qc

                    for h in range(4):
                        pv = pvp.tile([P, QC], f32, tag="pv", name=f"{I}pv{qc}_{h}")
                        exs = esp.tile([P, QC], bf, tag="exs", name=f"{I}exs{qc}_{h}")

                        pend = pending.pop((qc, h), {})
                        exs_init = set()  # exs halves already written
                        oid = 0
                        for oi, op in enumerate(ops):
                            kind, pl = op
                            ex = pend.get(oi)
                            if ex is None:
                                ex = emit_qk(qc, h, oi, op)
                            w = QC if kind == "full_pair" else HC
                            for j, item in enumerate(pl):
                                exj = ex[:, j * w:(j + 1) * w]
                                st = oid == 0
                                sp_ = oid == n_acc - 1
                                if kind == "full_pair":
                                    kt = item
                                    pv_reg = pv[:]
                                    exs_reg = exs[:]
                                else:
                                    kt, h2, dh = item
                                    pv_reg = pv[:, h2 * HC:(h2 + 1) * HC]
                                    exs_reg = exs[:, h2 * HC:(h2 + 1) * HC]
                                nc.tensor.matmul(
                                    pv_reg,
                                    v_nat[:, kt * P:(kt + 1) * P],
                                    exj,
                                    start=st,
                                    stop=sp_,
                                )
                                halves = {0, 1} if kind == "full_pair" else {h2}
                                if not (halves & exs_init):
                                    # first touch of these columns: copy
                                    nc.vector.tensor_copy(exs_reg, exj)
                                    exs_init |= halves
                                else:
                                    assert halves <= exs_init, (qc, h, halves)
                                    nc.vector.tensor_add(exs_reg, exs_reg, exj)
                                oid += 1
                            g += 1
                            # interleave out-proj pieces of the previous chunk
                            if qc > 0:
                                due = g * N_E // n_ops_qc
                                while pieces_done < due and pieces_done < N_E:
                                    emit_op_piece(qc - 1, pieces_done, pieces_done)
                                    pieces_done += 1

                        # prefetch next head's (or next chunk's) first QK+exp
                        # before the den/normalize chain so PE never waits.
                        if h < 3:
                            nxt = _ops_for(qc)
                            pending[(qc, h + 1)] = {0: emit_qk(qc, h + 1, 0, nxt[0])}
                        elif qc + 1 < N_QC:
                            nxt = _ops_for(qc + 1)
                            pending[(qc + 1, 0)] = {0: emit_qk(qc + 1, 0, 0, nxt[0])}

                        den = dbp.tile([1, QC], f32, tag="den", name=f"{I}den{qc}_{h}")
                        nc.tensor.matmul(
                            den[:], ones_col[:], exs[:], start=True, stop=True
                        )
                        recip = sp.tile([1, QC], f32, tag="recip", name=f"{I}rc{qc}_{h}")
                        nc.vector.reciprocal(recip[:], den[:])
                        bc_sb = sp.tile([P, QC], f32, tag="bc_sb", name=f"{I}bcs{qc}_{h}")
                        nc.gpsimd.partition_broadcast(bc_sb[:], recip[:])
                        nc.vector.tensor_mul(outT[h][:, qsl], pv[:], bc_sb[:])

                    # leftover pieces of qc-1 (qc0 has fewer ops than pieces)
                    if qc > 0:
                        while pieces_done < N_E:
                            emit_op_piece(qc - 1, pieces_done, pieces_done)
                            pieces_done += 1

                # final chunk's out-projection: attention is done, so the
                # sc slots are free -- rotate double-wide tiles through them
                # with alternating ACT/DVE evictions.
                qsl_l = slice((N_QC - 1) * QC, N_QC * QC)
                for ep_i in range(N_E // 2):
                    yp = scp.tile(
                        [P, 2 * QC], f32, tag="sc", name=f"{I}ypl{ep_i}"
                    )
                    for half in range(2):
                        e = 2 * ep_i + half
                        for d in range(4):
                            nc.tensor.matmul(
                                yp[:, half * QC:(half + 1) * QC],
                                wo_r[d][:, e * P:(e + 1) * P],
                                outT[d][:, qsl_l],
                                start=(d == 0),
                                stop=(d == 3),
                            )
                    y2 = sp.tile(
                        [P, 2 * QC], f32, tag="y_sb2", name=f"{I}ysbl{ep_i}"
                    )
                    if ep_i % 2 == 0:
                        nc.scalar.copy(y2[:], yp[:])
                    else:
                        nc.vector.tensor_copy(y2[:], yp[:])
                    nc.sync.dma_start(
                        out=y[2 * ep_i * P:(2 * ep_i + 2) * P, qsl_l].rearrange(
                            "(b p) q -> p b q", b=2
                        ),
                        in_=y2[:].rearrange("p (b q) -> p b q", b=2),
                    )

    nc.compile()
    return nc


def make_host_masks():
    import ml_dtypes

    m = np.zeros((len(MASK_DELTAS), P, QC), dtype=np.float32)
    ki = np.arange(P)[:, None]
    qi = np.arange(QC)[None, :]
    for i, d in enumerate(MASK_DELTAS):
        dist = d + qi - ki
        m[i] = ((dist >= 0) & (dist < WINDOW)).astype(np.float32)
    return m.astype(ml_dtypes.bfloat16)


def make_in_maps(x, cos, sin, Wq, Wk, Wv, Wo):
    import ml_dtypes

    bf = ml_dtypes.bfloat16
    cosT = np.ascontiguousarray(cos[:, 0, :].T)  # [128, S]
    sinT = sin[:, 0, :].T
    sinFT = np.concatenate([-sinT[: HD // 2], sinT[HD // 2:]], axis=0)
    sinFT = np.ascontiguousarray(sinFT.astype(np.float32))
    masks = make_host_masks()
    in_maps = []
    for c in range(8):
        b, g = c // 4, c % 4
        wq_g = Wq[g * 4 * HD:(g + 1) * 4 * HD, :]  # [512, E]
        wk_g = Wk[g * HD:(g + 1) * HD, :]  # [128, E]
        wv_g = Wv[g * HD:(g + 1) * HD, :]
        wqkvT = np.ascontiguousarray(
            np.concatenate([wq_g, wk_g, wv_g], axis=0).T
        ).astype(bf)  # [E, 768]
        woT_g = np.ascontiguousarray(
            Wo[:, g * 4 * HD:(g + 1) * 4 * HD].T
        ).astype(bf)  # [512, E]
        in_maps.append(
            {
                "xT": np.ascontiguousarray(x[b].T).astype(bf),
                "wqkvT": wqkvT,
                "woT": woT_g,
                "cosT": cosT,
                "sinFT": sinFT,
                "masks": masks,
            }
        )
    return in_maps


_NC_CACHE = {}


def get_nc():
    if "nc" not in _NC_CACHE:
        _NC_CACHE["nc"] = build_nc()
    return _NC_CACHE["nc"]


def kernel(x, cos, sin, Wq, Wk, Wv, Wo):
    from concourse.bass_utils import run_bass_kernel_spmd

    x = np.asarray(x, dtype=np.float32)
    cos = np.asarray(cos, dtype=np.float32)
    sin = np.asarray(sin, dtype=np.float32)
    Wq = np.asarray(Wq, dtype=np.float32)
    Wk = np.asarray(Wk, dtype=np.float32)
    Wv = np.asarray(Wv, dtype=np.float32)
    Wo = np.asarray(Wo, dtype=np.float32)

    nc = get_nc()
    in_maps = make_in_maps(x, cos, sin, Wq, Wk, Wv, Wo)
    res = run_bass_kernel_spmd(nc, in_maps, core_ids=list(range(8)))
    out = np.zeros((B, S, E), dtype=np.float32)
    for c in range(8):
        b = c // 4
        out[b] += res.results[c]["y"].T
    return out


# revision 13
# speedup vs baseline: 2.0927x; 2.0927x over previous
"""Trainium2 Bass kernel for CausalSelfAttention (GQA + RoPE + sliding window).

v5: bf16 data path (fp32 PSUM accumulation + fp32 RoPE + fp32 output),
per-(qc,h) softmax denominator via one ones-matmul over a DVE-accumulated
exp sum (drops the per-k-tile den matmuls from PE), out-projection
HALF-pieces (2 of 4 d-matmuls) interleaved into the NEXT q-chunk's
attention so every ACT exp-wait slot gets a ~426ns PE filler, f-outer/
e-inner projection chains for chunks s1..s3 so each feature's RoPE
eviction pipelines under the next feature's matmuls (s0 stays e-outer
for DMA-fill friendliness), and a short end-of-kernel drain (final
out-proj tiles rotate through both free PSUM pools with alternating
ACT/DVE evictions).

Module: B=2, S=2048, E=2048, NH=16 heads, NKV=4 kv heads, HD=128,
WINDOW=1024 (local causal: 0 <= q-k < 1024), fp32 in/out.

Sharding (8 cores): core = b*4 + g  where b = batch (2), g = kv-head group
(4). Each core handles 1 batch x 1 kv head (4 q heads); host sums the 4
partial out-projections per batch.

Layouts: xT [E,S] bf16, qT/kT [128,S] bf16 (RoPE computed in fp32 during
PSUM eviction, stored bf16), v natural [S,128] bf16 via PE transposes,
scoresT [k,q] fp32 PSUM -> exp -> ex bf16, exp row-sum accumulated in
bf16 SBUF (exs), den = ones^T @ exs, yT [E,S] fp32 partial output.
"""

import math
import os

import numpy as np

B, S, E = 2, 2048, 2048
NH, NKV, HD = 16, 4, 128
WINDOW = 1024
P = 128
QC = 512  # q chunk (moving free dim)
HC = 256  # half chunk for partial tiles
N_QC = S // QC  # 4
N_E = E // P  # 16 contraction chunks
SCALE = 1.0 / math.sqrt(HD)

# mask deltas: delta = q0 - 128*kt for partially-masked [k=128, q] tiles.
MASK_DELTAS = [-384, -256, -128, 0, 640, 768, 896, 1024]
MASK_IDX = {d: i for i, d in enumerate(MASK_DELTAS)}


def _kt_range(qc):
    kt_lo = max(0, (qc * QC - (WINDOW - 1)) // P)
    kt_hi = (qc * QC + QC - 1) // P
    return list(range(kt_lo, kt_hi + 1))


def _full_partial(qc):
    full, units = [], []
    for kt in _kt_range(qc):
        d = QC * qc - P * kt
        if 128 <= d <= 512:
            full.append(kt)
            continue
        for h2 in range(2):
            dh = d + h2 * HC
            lo, hi = dh - (P - 1), dh + (HC - 1)
            if hi < 0 or lo >= WINDOW:
                continue  # fully masked
            if lo >= 0 and hi < WINDOW:
                units.append((kt, h2, None))  # fully valid
            else:
                assert dh in MASK_IDX, (qc, kt, h2, dh)
                units.append((kt, h2, dh))
    return full, units


def _ops_for(qc):
    full_kts, units = _full_partial(qc)
    ops = []
    for i in range(0, len(full_kts), 2):
        ops.append(("full_pair", full_kts[i:i + 2]))
    for i in range(0, len(units), 2):
        ops.append(("unit_pair", units[i:i + 2]))
    return ops


def build_nc():
    import concourse.bass as bass
    import concourse.mybir as mybir
    import concourse.tile as tile
    from concourse import bacc
    from concourse.masks import make_identity

    f32 = mybir.dt.float32
    f32r = mybir.dt.float32r
    bf = mybir.dt.bfloat16
    Exp = mybir.ActivationFunctionType.Exp

    nc = bacc.Bacc("TRN2", target_bir_lowering=False, debug=False, num_devices=8)

    xT = nc.dram_tensor("xT", [E, S], bf, kind="ExternalInput")
    wqkvT = nc.dram_tensor("wqkvT", [E, 768], bf, kind="ExternalInput")
    woT = nc.dram_tensor("woT", [4 * P, E], bf, kind="ExternalInput")
    cosT = nc.dram_tensor("cosT", [P, S], f32, kind="ExternalInput")
    sinFT = nc.dram_tensor("sinFT", [P, S], f32, kind="ExternalInput")
    masks = nc.dram_tensor(
        "masks", [len(MASK_DELTAS), P, QC], bf, kind="ExternalInput"
    )
    y = nc.dram_tensor("y", [E, S], f32, kind="ExternalOutput")  # yT layout

    loop = int(os.environ.get("BASS_BENCH_LOOP", "1"))

    with tile.TileContext(nc) as tc:
        with (
            tc.tile_pool(name="persist", bufs=1) as pp,
            tc.tile_pool(name="wo_pool", bufs=1) as wop,
        ):
          for it in range(loop):
            I = f"i{it}_"
            qT_r = [pp.tile([P, S], bf, tag=f"qT{h}", name=f"{I}qT{h}") for h in range(4)]
            kT_r = pp.tile([P, S], bf, tag="kT", name=I + "kT")
            v_nat = pp.tile([P, S], bf, tag="v_nat", name=I + "v_nat")
            ident = pp.tile([P, P], f32, tag="ident", name=I + "ident")
            make_identity(nc, ident[:])
            ident_bf = pp.tile([P, P], bf, tag="ident_bf", name=I + "ident_bf")
            nc.vector.tensor_copy(ident_bf[:], ident[:])
            ones_col_f = pp.tile([P, 1], f32, tag="ones_col_f", name=I + "ones_col_f")
            ones_col = pp.tile([P, 1], bf, tag="ones_col", name=I + "ones_col")
            nc.vector.memset(ones_col_f[:], 1.0)
            nc.vector.tensor_copy(ones_col[:], ones_col_f[:])

            # ---------------- Phase 1: QKV projections + RoPE + v transpose
            with (
                tc.tile_pool(name=I + "wqkv_pool", bufs=1) as wqp,
                tc.tile_pool(name=I + "xpool", bufs=12) as xp,
                tc.tile_pool(name=I + "cspool", bufs=2) as csp,
                tc.tile_pool(name=I + "vstage", bufs=2) as vsp,
                tc.tile_pool(name=I + "proj_ps", bufs=1, space="PSUM") as pps,
                tc.tile_pool(name=I + "vtr_ps", bufs=1, space="PSUM") as vtps,
            ):
                # paired DMAs: 2 e-blocks per transfer
                wqkv2 = []
                x_pre = {}
                for ep2 in range(N_E // 2):
                    t = wqp.tile([P, 2 * 768], bf, tag=f"wqkv{ep2}", name=f"{I}wqkv{ep2}")
                    nc.sync.dma_start(
                        out=t[:].rearrange("p (b f) -> p b f", b=2),
                        in_=wqkvT[ep2 * 2 * P:(ep2 + 1) * 2 * P, :].rearrange(
                            "(b p) f -> p b f", b=2
                        ),
                    )
                    wqkv2.append(t)
                    x2 = xp.tile(
                        [P, 2 * QC], bf, tag="x_r", bufs=12, name=f"{I}x_r0_{ep2}"
                    )
                    nc.scalar.dma_start(
                        out=x2[:].rearrange("p (b q) -> p b q", b=2),
                        in_=xT[ep2 * 2 * P:(ep2 + 1) * 2 * P, 0:QC].rearrange(
                            "(b p) q -> p b q", b=2
                        ),
                    )
                    x_pre[(0, ep2)] = x2
                wqkv_sl = [
                    wqkv2[e // 2][:, (e % 2) * 768:(e % 2 + 1) * 768]
                    for e in range(N_E)
                ]

                cos_all = csp.tile([P, S], f32, tag="cos_all", bufs=1, name=I + "cos_all")
                sinF_all = csp.tile([P, S], f32, tag="sinF_all", bufs=1, name=I + "sinF_all")
                nc.scalar.dma_start(out=cos_all[:], in_=cosT[:])
                nc.scalar.dma_start(out=sinF_all[:], in_=sinFT[:])

                for s in range(N_QC):
                    ssl = slice(s * QC, (s + 1) * QC)
                    cos_sb = cos_all[:, ssl]
                    sinF_sb = sinF_all[:, ssl]

                    ps = [
                        pps.tile(
                            [P, QC], f32,
                            tag=f"proj{(f + s) % 7}",
                            name=f"{I}proj{f}_{s}",
                        )
                        for f in range(6)
                    ]
                    x_sl = {}
                    for ep2 in range(N_E // 2):
                        if (s, ep2) in x_pre:
                            x2 = x_pre[(s, ep2)]
                        else:
                            x2 = xp.tile(
                                [P, 2 * QC], bf, tag="x_r", bufs=12,
                                name=f"{I}x_r{s}_{ep2}",
                            )
                            nc.scalar.dma_start(
                                out=x2[:].rearrange("p (b q) -> p b q", b=2),
                                in_=xT[ep2 * 2 * P:(ep2 + 1) * 2 * P, ssl].rearrange(
                                    "(b p) q -> p b q", b=2
                                ),
                            )
                        x_sl[2 * ep2] = x2[:, 0:QC]
                        x_sl[2 * ep2 + 1] = x2[:, QC:2 * QC]

                    # s0: e-outer (each step needs only tile e -- friendly
                    # to the initial DMA fill). s1+: f-outer/e-inner so each
                    # feature's PSUM finishes early and its RoPE eviction
                    # pipelines under the next feature's matmul chain.
                    def fchain(f):
                        if s == 0 and f == 1:
                            # emit the whole chunk e-outer on the first call;
                            # later fchain calls for s0 are no-ops
                            for e in range(N_E):
                                for f_ in range(6):
                                    nc.tensor.matmul(
                                        ps[f_][:],
                                        wqkv_sl[e][:, f_ * P:(f_ + 1) * P],
                                        x_sl[e],
                                        start=(e == 0),
                                        stop=(e == N_E - 1),
                                    )
                        elif s == 0:
                            return
                        else:
                            for e in range(N_E):
                                nc.tensor.matmul(
                                    ps[f][:],
                                    wqkv_sl[e][:, f * P:(f + 1) * P],
                                    x_sl[e],
                                    start=(e == 0),
                                    stop=(e == N_E - 1),
                                )

                    # RoPE on fp32, stored bf16. Evict in the order the next
                    # chunk needs PSUM slots: q1,q2,q3,k,v,q0.
                    def rope_evict(dst, psrc, tmp_name):
                        stage = xp.tile(
                            [P, QC], f32, tag="rstage", bufs=3, name="st" + tmp_name
                        )
                        nc.scalar.copy(stage[:], psrc)
                        shf = xp.tile([P, QC], f32, tag="rope_shf", name="sh" + tmp_name)
                        H = P // 2
                        nc.vector.tensor_copy(shf[0:H, :], stage[H:P, :])
                        nc.vector.tensor_copy(shf[H:P, :], stage[0:H, :])
                        nc.vector.tensor_mul(shf[:], shf[:], sinF_sb)
                        nc.vector.tensor_mul(stage[:], stage[:], cos_sb)
                        nc.vector.tensor_add(dst, stage[:], shf[:])

                    for h in (1, 2, 3):
                        fchain(h)
                        rope_evict(qT_r[h][:, ssl], ps[h][:], f"{I}rope_q{h}_{s}")
                    fchain(4)
                    rope_evict(kT_r[:, ssl], ps[4][:], f"{I}rope_k{s}")
                    fchain(5)
                    v_sb = vsp.tile([P, QC], bf, tag="v_sb", name=f"{I}v_sb{s}")
                    nc.scalar.copy(v_sb[:], ps[5][:])
                    fchain(0)
                    rope_evict(qT_r[0][:, ssl], ps[0][:], f"{I}rope_q0_{s}")
                    for j in range(QC // P):
                        kt = s * (QC // P) + j
                        tps = vtps.tile([P, P], bf, tag="vtr", name=f"{I}vtr{kt}")
                        nc.tensor.transpose(
                            tps[:], v_sb[:, j * P:(j + 1) * P], ident_bf[:]
                        )
                        if s == N_QC - 1:
                            # keep ACT free for the first attention exps
                            nc.vector.tensor_copy(
                                v_nat[:, kt * P:(kt + 1) * P], tps[:]
                            )
                        else:
                            nc.scalar.copy(v_nat[:, kt * P:(kt + 1) * P], tps[:])

            # Wo resident load
            wo_r = []
            for d in range(4):
                t = wop.tile([P, E], bf, tag=f"wo_r{d}", name=f"{I}wo_r{d}")
                nc.sync.dma_start(out=t[:], in_=woT[d * P:(d + 1) * P, :])
                wo_r.append(t)

            # ---------------- Phase 2+3: attention with interleaved out-proj
            with (
                tc.tile_pool(name=I + "mask_pool", bufs=1) as mp,
                tc.tile_pool(name=I + "exp_pool", bufs=6) as ep,
                tc.tile_pool(name=I + "exs_pool", bufs=2) as esp,
                tc.tile_pool(name=I + "outT_pool", bufs=1) as op_,
                tc.tile_pool(name=I + "small_pool", bufs=3) as sp,
                tc.tile_pool(name=I + "sc_ps", bufs=2, space="PSUM") as scp,
                tc.tile_pool(name=I + "pv_ps", bufs=2, space="PSUM") as pvp,
                tc.tile_pool(name=I + "den_ps", bufs=1, space="PSUM") as dbp,
                tc.tile_pool(name=I + "op_ps", bufs=1, space="PSUM") as opp,
            ):
                nmask = len(MASK_DELTAS)
                mask_all = mp.tile(
                    [P, nmask * QC], bf, tag="mask_all", name=I + "mask_all"
                )
                nc.sync.dma_start(
                    out=mask_all[:].rearrange("p (m q) -> p m q", m=nmask),
                    in_=masks[:].rearrange("m p q -> p m q"),
                )
                mask_sb = [
                    mask_all[:, m * QC:(m + 1) * QC] for m in range(nmask)
                ]

                outT = [
                    op_.tile([P, S], bf, tag=f"outT{h}", name=f"{I}outT{h}")
                    for h in range(4)
                ]

                def emit_qk(qc, h, oi, op):
                    """QK matmuls + exp (+ mask muls) for one op; returns ex."""
                    kind, pl = op
                    qsl_ = slice(qc * QC, (qc + 1) * QC)
                    w = QC if kind == "full_pair" else HC
                    sc = scp.tile(
                        [P, 2 * QC], f32, tag="sc", name=f"{I}sc{qc}_{h}_{oi}"
                    )
                    for j, item in enumerate(pl):
                        if kind == "full_pair":
                            kt = item
                            qs = qsl_
                        else:
                            kt, h2, dh = item
                            q0 = qc * QC + h2 * HC
                            qs = slice(q0, q0 + HC)
                        nc.tensor.matmul(
                            sc[:, j * w:(j + 1) * w],
                            kT_r[:, kt * P:(kt + 1) * P],
                            qT_r[h][:, qs],
                            start=True,
                            stop=True,
                        )
                    ex = ep.tile(
                        [P, 2 * QC], bf, tag="ex", name=f"{I}ex{qc}_{h}_{oi}"
                    )
                    nc.scalar.activation(
                        ex[:, : len(pl) * w],
                        sc[:, : len(pl) * w],
                        Exp,
                        scale=SCALE,
                    )
                    if kind == "unit_pair":
                        for j, (kt, h2, dh) in enumerate(pl):
                            if dh is not None:
                                nc.vector.tensor_mul(
                                    ex[:, j * w:(j + 1) * w],
                                    ex[:, j * w:(j + 1) * w],
                                    mask_sb[MASK_IDX[dh]][:, :HC],
                                )
                    return ex

                op_yp = {}  # open half-piece psum tiles, keyed by e

                def emit_op_half(qc_prev, half_idx):
                    """Half an out-projection e-block (2 of 4 d-matmuls).

                    Splitting gives every attention op a ~426ns PE filler
                    instead of every other op a ~852ns one, matching the
                    ACT exp rate more evenly."""
                    e, hf = half_idx // 2, half_idx % 2
                    qsl_p = slice(qc_prev * QC, (qc_prev + 1) * QC)
                    if hf == 0:
                        op_yp[e] = opp.tile(
                            [P, QC], f32, tag="yp", name=f"{I}yp{qc_prev}_{e}"
                        )
                    yp = op_yp[e]
                    for d in (2 * hf, 2 * hf + 1):
                        nc.tensor.matmul(
                            yp[:],
                            wo_r[d][:, e * P:(e + 1) * P],
                            outT[d][:, qsl_p],
                            start=(d == 0),
                            stop=(d == 3),
                        )
                    if hf == 1:
                        del op_yp[e]
                        y2 = sp.tile(
                            [P, QC], f32, tag="y_sb", name=f"{I}ysb{qc_prev}_{e}"
                        )
                        if e % 2 == 0:
                            nc.scalar.copy(y2[:], yp[:])
                        else:
                            nc.vector.tensor_copy(y2[:], yp[:])
                        nc.sync.dma_start(
                            out=y[e * P:(e + 1) * P, qsl_p], in_=y2[:]
                        )

                pending = {}
                for qc in range(N_QC):
                    qsl = slice(qc * QC, (qc + 1) * QC)
                    ops = _ops_for(qc)
                    n_acc = sum(len(pl) for _, pl in ops)
                    n_ops_qc = 4 * len(ops)
                    pieces_done = 0
                    g = 0  # global op index within this qc

                    for h in range(4):
                        pv = pvp.tile([P, QC], f32, tag="pv", name=f"{I}pv{qc}_{h}")
                        exs = esp.tile([P, QC], bf, tag="exs", name=f"{I}exs{qc}_{h}")

                        pend = pending.pop((qc, h), {})
                        exs_init = set()  # exs halves already written
                        oid = 0
                        for oi, op in enumerate(ops):
                            kind, pl = op
                            ex = pend.get(oi)
                            if ex is None:
                                ex = emit_qk(qc, h, oi, op)
                            w = QC if kind == "full_pair" else HC
                            for j, item in enumerate(pl):
                                exj = ex[:, j * w:(j + 1) * w]
                                st = oid == 0
                                sp_ = oid == n_acc - 1
                                if kind == "full_pair":
                                    kt = item
                                    pv_reg = pv[:]
                                    exs_reg = exs[:]
                                else:
                                    kt, h2, dh = item
                                    pv_reg = pv[:, h2 * HC:(h2 + 1) * HC]
                                    exs_reg = exs[:, h2 * HC:(h2 + 1) * HC]
                                nc.tensor.matmul(
                                    pv_reg,
                                    v_nat[:, kt * P:(kt + 1) * P],
                                    exj,
                                    start=st,
                                    stop=sp_,
                                )
                                halves = {0, 1} if kind == "full_pair" else {h2}
                                if not (halves & exs_init):
                                    # first touch of these columns: copy
                                    nc.vector.tensor_copy(exs_reg, exj)
                                    exs_init |= halves
                                else:
                                    assert halves <= exs_init, (qc, h, halves)
                                    nc.vector.tensor_add(exs_reg, exs_reg, exj)
                                oid += 1
                            g += 1
                            # interleave out-proj half-pieces of the previous
                            # chunk as PE filler for the ACT exp-wait slots
                            if qc > 0:
                                due = g * (2 * N_E) // n_ops_qc
                                while pieces_done < due and pieces_done < 2 * N_E:
                                    emit_op_half(qc - 1, pieces_done)
                                    pieces_done += 1

                        # prefetch next head's (or next chunk's) first QK+exp
                        # before the den/normalize chain so PE never waits.
                        if h < 3:
                            nxt = _ops_for(qc)
                            pending[(qc, h + 1)] = {0: emit_qk(qc, h + 1, 0, nxt[0])}
                        elif qc + 1 < N_QC:
                            nxt = _ops_for(qc + 1)
                            pending[(qc + 1, 0)] = {0: emit_qk(qc + 1, 0, 0, nxt[0])}

                        den = dbp.tile([1, QC], f32, tag="den", name=f"{I}den{qc}_{h}")
                        nc.tensor.matmul(
                            den[:], ones_col[:], exs[:], start=True, stop=True
                        )
                        recip = sp.tile([1, QC], f32, tag="recip", name=f"{I}rc{qc}_{h}")
                        nc.vector.reciprocal(recip[:], den[:])
                        bc_sb = sp.tile([P, QC], f32, tag="bc_sb", name=f"{I}bcs{qc}_{h}")
                        nc.gpsimd.partition_broadcast(bc_sb[:], recip[:])
                        nc.vector.tensor_mul(outT[h][:, qsl], pv[:], bc_sb[:])

                    # leftover halves of qc-1 (qc0 has fewer ops than halves)
                    if qc > 0:
                        while pieces_done < 2 * N_E:
                            emit_op_half(qc - 1, pieces_done)
                            pieces_done += 1

                # final chunk's out-projection: attention is done, so the
                # sc slots are free -- rotate double-wide tiles through them
                # with alternating ACT/DVE evictions.
                qsl_l = slice((N_QC - 1) * QC, N_QC * QC)
                for ep_i in range(N_E // 2 - 2):
                    yp = scp.tile(
                        [P, 2 * QC], f32, tag="sc", name=f"{I}ypl{ep_i}"
                    )
                    for half in range(2):
                        e = 2 * ep_i + half
                        for d in range(4):
                            nc.tensor.matmul(
                                yp[:, half * QC:(half + 1) * QC],
                                wo_r[d][:, e * P:(e + 1) * P],
                                outT[d][:, qsl_l],
                                start=(d == 0),
                                stop=(d == 3),
                            )
                    y2 = sp.tile(
                        [P, 2 * QC], f32, tag="y_sb2", name=f"{I}ysbl{ep_i}"
                    )
                    if ep_i % 2 == 0:
                        nc.scalar.copy(y2[:], yp[:])
                    else:
                        nc.vector.tensor_copy(y2[:], yp[:])
                    nc.sync.dma_start(
                        out=y[2 * ep_i * P:(2 * ep_i + 2) * P, qsl_l].rearrange(
                            "(b p) q -> p b q", b=2
                        ),
                        in_=y2[:].rearrange("p (b q) -> p b q", b=2),
                    )
                # last 4 e-blocks as single-wide pieces: smaller eviction and
                # DMA quanta shorten the end-of-kernel drain
                for li, e in enumerate(range(N_E - 4, N_E)):
                    if li % 2 == 0:
                        ypl = opp.tile([P, QC], f32, tag="yp", name=f"{I}ypt{e}")
                    else:
                        ypl = scp.tile([P, 2 * QC], f32, tag="sc", name=f"{I}ypt{e}")

                    ypv = ypl[:, 0:QC]
                    for d in range(4):
                        nc.tensor.matmul(
                            ypv,
                            wo_r[d][:, e * P:(e + 1) * P],
                            outT[d][:, qsl_l],
                            start=(d == 0),
                            stop=(d == 3),
                        )
                    y2l = sp.tile([P, QC], f32, tag="y_sb", name=f"{I}ysbt{e}")
                    if li % 2 == 0:
                        nc.scalar.copy(y2l[:], ypv)
                    else:
                        nc.vector.tensor_copy(y2l[:], ypv)
                    nc.sync.dma_start(out=y[e * P:(e + 1) * P, qsl_l], in_=y2l[:])

    nc.compile()
    return nc


def make_host_masks():
    import ml_dtypes

    m = np.zeros((len(MASK_DELTAS), P, QC), dtype=np.float32)
    ki = np.arange(P)[:, None]
    qi = np.arange(QC)[None, :]
    for i, d in enumerate(MASK_DELTAS):
        dist = d + qi - ki
        m[i] = ((dist >= 0) & (dist < WINDOW)).astype(np.float32)
    return m.astype(ml_dtypes.bfloat16)


def make_in_maps(x, cos, sin, Wq, Wk, Wv, Wo):
    import ml_dtypes

    bf = ml_dtypes.bfloat16
    cosT = np.ascontiguousarray(cos[:, 0, :].T)  # [128, S]
    sinT = sin[:, 0, :].T
    sinFT = np.concatenate([-sinT[: HD // 2], sinT[HD // 2:]], axis=0)
    sinFT = np.ascontiguousarray(sinFT.astype(np.float32))
    masks = make_host_masks()
    in_maps = []
    for c in range(8):
        b, g = c // 4, c % 4
        wq_g = Wq[g * 4 * HD:(g + 1) * 4 * HD, :]  # [512, E]
        wk_g = Wk[g * HD:(g + 1) * HD, :]  # [128, E]
        wv_g = Wv[g * HD:(g + 1) * HD, :]
        wqkvT = np.ascontiguousarray(
            np.concatenate([wq_g, wk_g, wv_g], axis=0).T
        ).astype(bf)  # [E, 768]
        woT_g = np.ascontiguousarray(
            Wo[:, g * 4 * HD:(g + 1) * 4 * HD].T
        ).astype(bf)  # [512, E]
        in_maps.append(
            {
                "xT": np.ascontiguousarray(x[b].T).astype(bf),
                "wqkvT": wqkvT,
                "woT": woT_g,
                "cosT": cosT,
                "sinFT": sinFT,
                "masks": masks,
            }
        )
    return in_maps


_NC_CACHE = {}


def get_nc():
    if "nc" not in _NC_CACHE:
        _NC_CACHE["nc"] = build_nc()
    return _NC_CACHE["nc"]


def kernel(x, cos, sin, Wq, Wk, Wv, Wo):
    from concourse.bass_utils import run_bass_kernel_spmd

    x = np.asarray(x, dtype=np.float32)
    cos = np.asarray(cos, dtype=np.float32)
    sin = np.asarray(sin, dtype=np.float32)
    Wq = np.asarray(Wq, dtype=np.float32)
    Wk = np.asarray(Wk, dtype=np.float32)
    Wv = np.asarray(Wv, dtype=np.float32)
    Wo = np.asarray(Wo, dtype=np.float32)

    nc = get_nc()
    in_maps = make_in_maps(x, cos, sin, Wq, Wk, Wv, Wo)
    res = run_bass_kernel_spmd(nc, in_maps, core_ids=list(range(8)))
    out = np.zeros((B, S, E), dtype=np.float32)
    for c in range(8):
        b = c // 4
        out[b] += res.results[c]["y"].T
    return out


# revision 23
# speedup vs baseline: 2.7308x; 1.3049x over previous
"""Trainium2 Bass kernel for CausalSelfAttention (GQA + RoPE + sliding window).

v6: bf16 data path (fp32 PSUM accumulation + fp32 RoPE + fp32 output),
per-(qc,h) softmax denominator via one ones-matmul over a DVE-accumulated
exp sum (drops the per-k-tile den matmuls from PE), out-projection
HALF-pieces (2 of 4 d-matmuls) interleaved into the NEXT q-chunk's
attention so every ACT exp-wait slot gets a ~426ns PE filler,
partially-masked k-tiles whose two halves are (masked, valid) merged
into single 512-wide ops with the full-width mask (25% fewer attention
ops, ~18us less ACT exp time), f-outer/
e-inner projection chains for chunks s1..s3 so each feature's RoPE
eviction pipelines under the next feature's matmuls (s0 stays e-outer
for DMA-fill friendliness), and a short end-of-kernel drain (final
out-proj tiles rotate through both free PSUM pools with alternating
ACT/DVE evictions).

Module: B=2, S=2048, E=2048, NH=16 heads, NKV=4 kv heads, HD=128,
WINDOW=1024 (local causal: 0 <= q-k < 1024), fp32 in/out.

Sharding (8 cores): core = b*4 + g  where b = batch (2), g = kv-head group
(4). Each core handles 1 batch x 1 kv head (4 q heads); host sums the 4
partial out-projections per batch.

Layouts: xT [E,S] bf16, qT/kT [128,S] bf16 (RoPE computed in fp32 during
PSUM eviction, stored bf16), v natural [S,128] bf16 via PE transposes,
scoresT [k,q] fp32 PSUM -> exp -> ex bf16, exp row-sum accumulated in
bf16 SBUF (exs), den = ones^T @ exs, yT [E,S] fp32 partial output.
"""

import math
import os

import numpy as np

B, S, E = 2, 2048, 2048
NH, NKV, HD = 16, 4, 128
WINDOW = 1024
P = 128
QC = 512  # q chunk (moving free dim)
HC = 256  # half chunk for partial tiles
N_QC = S // QC  # 4
N_E = E // P  # 16 contraction chunks
SCALE = 1.0 / math.sqrt(HD)

# mask deltas: delta = q0 - 128*kt for partially-masked [k=128, q] tiles.
MASK_DELTAS = [-384, -256, -128, 0, 640, 768, 896, 1024]
MASK_IDX = {d: i for i, d in enumerate(MASK_DELTAS)}


def _kt_range(qc):
    kt_lo = max(0, (qc * QC - (WINDOW - 1)) // P)
    kt_hi = (qc * QC + QC - 1) // P
    return list(range(kt_lo, kt_hi + 1))


def _full_partial(qc):
    """Classify k-tiles: wide = 512-wide ops [(kt, mask_delta|None)],
    units = surviving 256-wide halves [(kt, h2, mask_delta)].

    A tile whose two halves are (masked, valid) or (valid, masked) is
    emitted as ONE 512-wide op with the full-width mask for delta d --
    the valid half's mask region is all-ones, so the multiply is exact."""
    wide, units = [], []
    for kt in _kt_range(qc):
        d = QC * qc - P * kt
        if 128 <= d <= 512:
            wide.append((kt, None))
            continue
        halves = []
        for h2 in range(2):
            dh = d + h2 * HC
            lo, hi = dh - (P - 1), dh + (HC - 1)
            if hi < 0 or lo >= WINDOW:
                halves.append(None)  # fully masked
            elif lo >= 0 and hi < WINDOW:
                halves.append((kt, h2, None))  # fully valid
            else:
                assert dh in MASK_IDX, (qc, kt, h2, dh)
                halves.append((kt, h2, dh))
        alive = [h for h in halves if h is not None]
        if len(alive) == 2:
            assert d in MASK_IDX, (qc, kt, d)
            wide.append((kt, d))  # one 512-wide masked op
        elif len(alive) == 1:
            units.append(alive[0])
    return wide, units


def _ops_for(qc):
    wide, units = _full_partial(qc)
    ops = []
    for i in range(0, len(wide), 2):
        ops.append(("full_pair", wide[i:i + 2]))
    for i in range(0, len(units), 2):
        ops.append(("unit_pair", units[i:i + 2]))
    return ops


def build_nc():
    import concourse.bass as bass
    import concourse.mybir as mybir
    import concourse.tile as tile
    from concourse import bacc
    from concourse.masks import make_identity

    f32 = mybir.dt.float32
    f32r = mybir.dt.float32r
    bf = mybir.dt.bfloat16
    Exp = mybir.ActivationFunctionType.Exp

    nc = bacc.Bacc("TRN2", target_bir_lowering=False, debug=False, num_devices=8)

    xT = nc.dram_tensor("xT", [E, S], bf, kind="ExternalInput")
    wqkvT = nc.dram_tensor("wqkvT", [E, 768], bf, kind="ExternalInput")
    woT = nc.dram_tensor("woT", [4 * P, E], bf, kind="ExternalInput")
    cosT = nc.dram_tensor("cosT", [P, S], f32, kind="ExternalInput")
    sinFT = nc.dram_tensor("sinFT", [P, S], f32, kind="ExternalInput")
    masks = nc.dram_tensor(
        "masks", [len(MASK_DELTAS), P, QC], bf, kind="ExternalInput"
    )
    y = nc.dram_tensor("y", [E, S], f32, kind="ExternalOutput")  # yT layout

    loop = int(os.environ.get("BASS_BENCH_LOOP", "1"))

    with tile.TileContext(nc) as tc:
        with (
            tc.tile_pool(name="persist", bufs=1) as pp,
            tc.tile_pool(name="wo_pool", bufs=1) as wop,
        ):
          for it in range(loop):
            I = f"i{it}_"
            qT_r = [pp.tile([P, S], bf, tag=f"qT{h}", name=f"{I}qT{h}") for h in range(4)]
            kT_r = pp.tile([P, S], bf, tag="kT", name=I + "kT")
            v_nat = pp.tile([P, S], bf, tag="v_nat", name=I + "v_nat")
            ident = pp.tile([P, P], f32, tag="ident", name=I + "ident")
            make_identity(nc, ident[:])
            ident_bf = pp.tile([P, P], bf, tag="ident_bf", name=I + "ident_bf")
            nc.vector.tensor_copy(ident_bf[:], ident[:])
            ones_col_f = pp.tile([P, 1], f32, tag="ones_col_f", name=I + "ones_col_f")
            ones_col = pp.tile([P, 1], bf, tag="ones_col", name=I + "ones_col")
            nc.vector.memset(ones_col_f[:], 1.0)
            nc.vector.tensor_copy(ones_col[:], ones_col_f[:])

            # ---------------- Phase 1: QKV projections + RoPE + v transpose
            with (
                tc.tile_pool(name=I + "wqkv_pool", bufs=1) as wqp,
                tc.tile_pool(name=I + "xpool", bufs=12) as xp,
                tc.tile_pool(name=I + "cspool", bufs=2) as csp,
                tc.tile_pool(name=I + "vstage", bufs=2) as vsp,
                tc.tile_pool(name=I + "proj_ps", bufs=1, space="PSUM") as pps,
                tc.tile_pool(name=I + "vtr_ps", bufs=1, space="PSUM") as vtps,
            ):
                # paired DMAs: 2 e-blocks per transfer
                wqkv2 = []
                x_pre = {}
                for ep2 in range(N_E // 2):
                    t = wqp.tile([P, 2 * 768], bf, tag=f"wqkv{ep2}", name=f"{I}wqkv{ep2}")
                    nc.sync.dma_start(
                        out=t[:].rearrange("p (b f) -> p b f", b=2),
                        in_=wqkvT[ep2 * 2 * P:(ep2 + 1) * 2 * P, :].rearrange(
                            "(b p) f -> p b f", b=2
                        ),
                    )
                    wqkv2.append(t)
                    x2 = xp.tile(
                        [P, 2 * QC], bf, tag="x_r", bufs=12, name=f"{I}x_r0_{ep2}"
                    )
                    nc.scalar.dma_start(
                        out=x2[:].rearrange("p (b q) -> p b q", b=2),
                        in_=xT[ep2 * 2 * P:(ep2 + 1) * 2 * P, 0:QC].rearrange(
                            "(b p) q -> p b q", b=2
                        ),
                    )
                    x_pre[(0, ep2)] = x2
                wqkv_sl = [
                    wqkv2[e // 2][:, (e % 2) * 768:(e % 2 + 1) * 768]
                    for e in range(N_E)
                ]

                cos_all = csp.tile([P, S], f32, tag="cos_all", bufs=1, name=I + "cos_all")
                sinF_all = csp.tile([P, S], f32, tag="sinF_all", bufs=1, name=I + "sinF_all")
                nc.scalar.dma_start(out=cos_all[:], in_=cosT[:])
                nc.scalar.dma_start(out=sinF_all[:], in_=sinFT[:])

                for s in range(N_QC):
                    ssl = slice(s * QC, (s + 1) * QC)
                    cos_sb = cos_all[:, ssl]
                    sinF_sb = sinF_all[:, ssl]

                    ps = [
                        pps.tile(
                            [P, QC], f32,
                            tag=f"proj{(f + s) % 7}",
                            name=f"{I}proj{f}_{s}",
                        )
                        for f in range(6)
                    ]
                    x_sl = {}
                    for ep2 in range(N_E // 2):
                        if (s, ep2) in x_pre:
                            x2 = x_pre[(s, ep2)]
                        else:
                            x2 = xp.tile(
                                [P, 2 * QC], bf, tag="x_r", bufs=12,
                                name=f"{I}x_r{s}_{ep2}",
                            )
                            nc.scalar.dma_start(
                                out=x2[:].rearrange("p (b q) -> p b q", b=2),
                                in_=xT[ep2 * 2 * P:(ep2 + 1) * 2 * P, ssl].rearrange(
                                    "(b p) q -> p b q", b=2
                                ),
                            )
                        x_sl[2 * ep2] = x2[:, 0:QC]
                        x_sl[2 * ep2 + 1] = x2[:, QC:2 * QC]

                    # s0: e-outer (each step needs only tile e -- friendly
                    # to the initial DMA fill). s1+: f-outer/e-inner so each
                    # feature's PSUM finishes early and its RoPE eviction
                    # pipelines under the next feature's matmul chain.
                    def fchain(f):
                        if s == 0 and f == 1:
                            # emit the whole chunk e-outer on the first call;
                            # later fchain calls for s0 are no-ops
                            for e in range(N_E):
                                for f_ in range(6):
                                    nc.tensor.matmul(
                                        ps[f_][:],
                                        wqkv_sl[e][:, f_ * P:(f_ + 1) * P],
                                        x_sl[e],
                                        start=(e == 0),
                                        stop=(e == N_E - 1),
                                    )
                        elif s == 0:
                            return
                        else:
                            for e in range(N_E):
                                nc.tensor.matmul(
                                    ps[f][:],
                                    wqkv_sl[e][:, f * P:(f + 1) * P],
                                    x_sl[e],
                                    start=(e == 0),
                                    stop=(e == N_E - 1),
                                )

                    # RoPE on fp32, stored bf16. Evict in the order the next
                    # chunk needs PSUM slots: q1,q2,q3,k,v,q0.
                    def rope_evict(dst, psrc, tmp_name):
                        stage = xp.tile(
                            [P, QC], f32, tag="rstage", bufs=3, name="st" + tmp_name
                        )
                        nc.scalar.copy(stage[:], psrc)
                        shf = xp.tile([P, QC], f32, tag="rope_shf", name="sh" + tmp_name)
                        H = P // 2
                        nc.vector.tensor_copy(shf[0:H, :], stage[H:P, :])
                        nc.vector.tensor_copy(shf[H:P, :], stage[0:H, :])
                        nc.vector.tensor_mul(shf[:], shf[:], sinF_sb)
                        nc.vector.tensor_mul(stage[:], stage[:], cos_sb)
                        nc.vector.tensor_add(dst, stage[:], shf[:])

                    for h in (1, 2, 3):
                        fchain(h)
                        rope_evict(qT_r[h][:, ssl], ps[h][:], f"{I}rope_q{h}_{s}")
                    fchain(4)
                    rope_evict(kT_r[:, ssl], ps[4][:], f"{I}rope_k{s}")
                    fchain(5)
                    v_sb = vsp.tile([P, QC], bf, tag="v_sb", name=f"{I}v_sb{s}")
                    nc.scalar.copy(v_sb[:], ps[5][:])
                    fchain(0)
                    rope_evict(qT_r[0][:, ssl], ps[0][:], f"{I}rope_q0_{s}")
                    for j in range(QC // P):
                        kt = s * (QC // P) + j
                        tps = vtps.tile([P, P], bf, tag="vtr", name=f"{I}vtr{kt}")
                        nc.tensor.transpose(
                            tps[:], v_sb[:, j * P:(j + 1) * P], ident_bf[:]
                        )
                        if s == N_QC - 1:
                            # keep ACT free for the first attention exps
                            nc.vector.tensor_copy(
                                v_nat[:, kt * P:(kt + 1) * P], tps[:]
                            )
                        else:
                            nc.scalar.copy(v_nat[:, kt * P:(kt + 1) * P], tps[:])

            # Wo resident load
            wo_r = []
            for d in range(4):
                t = wop.tile([P, E], bf, tag=f"wo_r{d}", name=f"{I}wo_r{d}")
                nc.sync.dma_start(out=t[:], in_=woT[d * P:(d + 1) * P, :])
                wo_r.append(t)

            # ---------------- Phase 2+3: attention with interleaved out-proj
            with (
                tc.tile_pool(name=I + "mask_pool", bufs=1) as mp,
                tc.tile_pool(name=I + "exp_pool", bufs=8) as ep,
                tc.tile_pool(name=I + "exs_pool", bufs=3) as esp,
                tc.tile_pool(name=I + "outT_pool", bufs=1) as op_,
                tc.tile_pool(name=I + "small_pool", bufs=4) as sp,
                tc.tile_pool(name=I + "sc_ps", bufs=2, space="PSUM") as scp,
                tc.tile_pool(name=I + "pv_ps", bufs=2, space="PSUM") as pvp,
                tc.tile_pool(name=I + "den_ps", bufs=1, space="PSUM") as dbp,
                tc.tile_pool(name=I + "op_ps", bufs=1, space="PSUM") as opp,
            ):
                nmask = len(MASK_DELTAS)
                mask_all = mp.tile(
                    [P, nmask * QC], bf, tag="mask_all", name=I + "mask_all"
                )
                nc.sync.dma_start(
                    out=mask_all[:].rearrange("p (m q) -> p m q", m=nmask),
                    in_=masks[:].rearrange("m p q -> p m q"),
                )
                mask_sb = [
                    mask_all[:, m * QC:(m + 1) * QC] for m in range(nmask)
                ]

                outT = [
                    op_.tile([P, S], bf, tag=f"outT{h}", name=f"{I}outT{h}")
                    for h in range(4)
                ]

                def emit_qk(qc, h, oi, op):
                    """QK matmuls + exp (+ mask muls) for one op; returns ex."""
                    kind, pl = op
                    qsl_ = slice(qc * QC, (qc + 1) * QC)
                    w = QC if kind == "full_pair" else HC
                    sc = scp.tile(
                        [P, 2 * QC], f32, tag="sc", name=f"{I}sc{qc}_{h}_{oi}"
                    )
                    for j, item in enumerate(pl):
                        if kind == "full_pair":
                            kt, _md = item
                            qs = qsl_
                        else:
                            kt, h2, dh = item
                            q0 = qc * QC + h2 * HC
                            qs = slice(q0, q0 + HC)
                        nc.tensor.matmul(
                            sc[:, j * w:(j + 1) * w],
                            kT_r[:, kt * P:(kt + 1) * P],
                            qT_r[h][:, qs],
                            start=True,
                            stop=True,
                        )
                    ex = ep.tile(
                        [P, 2 * QC], bf, tag="ex", name=f"{I}ex{qc}_{h}_{oi}"
                    )
                    nc.scalar.activation(
                        ex[:, : len(pl) * w],
                        sc[:, : len(pl) * w],
                        Exp,
                        scale=SCALE,
                    )
                    if kind == "unit_pair":
                        for j, (kt, h2, dh) in enumerate(pl):
                            if dh is not None:
                                nc.vector.tensor_mul(
                                    ex[:, j * w:(j + 1) * w],
                                    ex[:, j * w:(j + 1) * w],
                                    mask_sb[MASK_IDX[dh]][:, :HC],
                                )
                    else:
                        for j, (kt, md) in enumerate(pl):
                            if md is not None:
                                nc.vector.tensor_mul(
                                    ex[:, j * w:(j + 1) * w],
                                    ex[:, j * w:(j + 1) * w],
                                    mask_sb[MASK_IDX[md]],
                                )
                    return ex

                op_yp = {}  # open half-piece psum tiles, keyed by e

                def emit_op_half(qc_prev, half_idx):
                    """Half an out-projection e-block (2 of 4 d-matmuls).

                    Splitting gives every attention op a ~426ns PE filler
                    instead of every other op a ~852ns one, matching the
                    ACT exp rate more evenly."""
                    e, hf = half_idx // 2, half_idx % 2
                    qsl_p = slice(qc_prev * QC, (qc_prev + 1) * QC)
                    if hf == 0:
                        op_yp[e] = opp.tile(
                            [P, QC], f32, tag="yp", name=f"{I}yp{qc_prev}_{e}"
                        )
                    yp = op_yp[e]
                    for d in (2 * hf, 2 * hf + 1):
                        nc.tensor.matmul(
                            yp[:],
                            wo_r[d][:, e * P:(e + 1) * P],
                            outT[d][:, qsl_p],
                            start=(d == 0),
                            stop=(d == 3),
                        )
                    if hf == 1:
                        del op_yp[e]
                        y2 = sp.tile(
                            [P, QC], f32, tag="y_sb", name=f"{I}ysb{qc_prev}_{e}"
                        )
                        if e % 2 == 0:
                            nc.scalar.copy(y2[:], yp[:])
                        else:
                            nc.vector.tensor_copy(y2[:], yp[:])
                        nc.sync.dma_start(
                            out=y[e * P:(e + 1) * P, qsl_p], in_=y2[:]
                        )

                pending = {}
                for qc in range(N_QC):
                    qsl = slice(qc * QC, (qc + 1) * QC)
                    ops = _ops_for(qc)
                    n_acc = sum(len(pl) for _, pl in ops)
                    n_ops_qc = 4 * len(ops)
                    pieces_done = 0
                    g = 0  # global op index within this qc

                    for h in range(4):
                        pv = pvp.tile([P, QC], f32, tag="pv", name=f"{I}pv{qc}_{h}")
                        exs = esp.tile([P, QC], bf, tag="exs", name=f"{I}exs{qc}_{h}")

                        pend = pending.pop((qc, h), {})
                        exs_init = set()  # exs halves already written
                        oid = 0
                        for oi, op in enumerate(ops):
                            kind, pl = op
                            ex = pend.get(oi)
                            if ex is None:
                                ex = emit_qk(qc, h, oi, op)
                            w = QC if kind == "full_pair" else HC
                            for j, item in enumerate(pl):
                                exj = ex[:, j * w:(j + 1) * w]
                                st = oid == 0
                                sp_ = oid == n_acc - 1
                                if kind == "full_pair":
                                    kt, _md = item
                                    pv_reg = pv[:]
                                    exs_reg = exs[:]
                                else:
                                    kt, h2, dh = item
                                    pv_reg = pv[:, h2 * HC:(h2 + 1) * HC]
                                    exs_reg = exs[:, h2 * HC:(h2 + 1) * HC]
                                nc.tensor.matmul(
                                    pv_reg,
                                    v_nat[:, kt * P:(kt + 1) * P],
                                    exj,
                                    start=st,
                                    stop=sp_,
                                )
                                halves = {0, 1} if kind == "full_pair" else {h2}
                                if not (halves & exs_init):
                                    # first touch of these columns: copy
                                    nc.vector.tensor_copy(exs_reg, exj)
                                    exs_init |= halves
                                else:
                                    assert halves <= exs_init, (qc, h, halves)
                                    nc.vector.tensor_add(exs_reg, exs_reg, exj)
                                oid += 1
                            g += 1
                            # interleave out-proj half-pieces of the previous
                            # chunk as PE filler for the ACT exp-wait slots
                            if qc > 0:
                                due = g * (2 * N_E) // n_ops_qc
                                while pieces_done < due and pieces_done < 2 * N_E:
                                    emit_op_half(qc - 1, pieces_done)
                                    pieces_done += 1

                        # prefetch next head's (or next chunk's) first QK+exp
                        # before the den/normalize chain so PE never waits.
                        if h < 3:
                            nxt = _ops_for(qc)
                            pending[(qc, h + 1)] = {0: emit_qk(qc, h + 1, 0, nxt[0])}
                        elif qc + 1 < N_QC:
                            nxt = _ops_for(qc + 1)
                            pending[(qc + 1, 0)] = {0: emit_qk(qc + 1, 0, 0, nxt[0])}

                        den = dbp.tile([1, QC], f32, tag="den", name=f"{I}den{qc}_{h}")
                        nc.tensor.matmul(
                            den[:], ones_col[:], exs[:], start=True, stop=True
                        )
                        recip = sp.tile([1, QC], f32, tag="recip", name=f"{I}rc{qc}_{h}")
                        # den = sums of positive exps (~1..1e4): no +-0/denorm/
                        # inf edge cases, and ~18-bit accuracy is far inside
                        # the error budget. Single DVE op, ~5x faster than
                        # InstReciprocal -- this sits on the per-head
                        # den->recip->broadcast->normalize latency chain.
                        nc.vector.reciprocal_approx_fast(recip[:], den[:])
                        bc_sb = sp.tile([P, QC], f32, tag="bc_sb", name=f"{I}bcs{qc}_{h}")
                        nc.gpsimd.partition_broadcast(bc_sb[:], recip[:])
                        nc.vector.tensor_mul(outT[h][:, qsl], pv[:], bc_sb[:])

                    # leftover halves of qc-1 (qc0 has fewer ops than halves)
                    if qc > 0:
                        while pieces_done < 2 * N_E:
                            emit_op_half(qc - 1, pieces_done)
                            pieces_done += 1

                # final chunk's out-projection: attention is done, so the
                # sc slots are free -- rotate double-wide tiles through them
                # with alternating ACT/DVE evictions.
                qsl_l = slice((N_QC - 1) * QC, N_QC * QC)
                for ep_i in range(N_E // 2 - 2):
                    yp = scp.tile(
                        [P, 2 * QC], f32, tag="sc", name=f"{I}ypl{ep_i}"
                    )
                    for half in range(2):
                        e = 2 * ep_i + half
                        for d in range(4):
                            nc.tensor.matmul(
                                yp[:, half * QC:(half + 1) * QC],
                                wo_r[d][:, e * P:(e + 1) * P],
                                outT[d][:, qsl_l],
                                start=(d == 0),
                                stop=(d == 3),
                            )
                    y2 = sp.tile(
                        [P, 2 * QC], f32, tag="y_sb2", name=f"{I}ysbl{ep_i}"
                    )
                    if ep_i % 2 == 0:
                        nc.scalar.copy(y2[:], yp[:])
                    else:
                        nc.vector.tensor_copy(y2[:], yp[:])
                    nc.sync.dma_start(
                        out=y[2 * ep_i * P:(2 * ep_i + 2) * P, qsl_l].rearrange(
                            "(b p) q -> p b q", b=2
                        ),
                        in_=y2[:].rearrange("p (b q) -> p b q", b=2),
                    )
                # last 4 e-blocks as single-wide pieces: smaller eviction and
                # DMA quanta shorten the end-of-kernel drain
                for li, e in enumerate(range(N_E - 4, N_E)):
                    if li % 2 == 0:
                        ypl = opp.tile([P, QC], f32, tag="yp", name=f"{I}ypt{e}")
                    else:
                        ypl = scp.tile([P, 2 * QC], f32, tag="sc", name=f"{I}ypt{e}")

                    ypv = ypl[:, 0:QC]
                    for d in range(4):
                        nc.tensor.matmul(
                            ypv,
                            wo_r[d][:, e * P:(e + 1) * P],
                            outT[d][:, qsl_l],
                            start=(d == 0),
                            stop=(d == 3),
                        )
                    y2l = sp.tile([P, QC], f32, tag="y_sb", name=f"{I}ysbt{e}")
                    if li % 2 == 0:
                        nc.scalar.copy(y2l[:], ypv)
                    else:
                        nc.vector.tensor_copy(y2l[:], ypv)
                    nc.sync.dma_start(out=y[e * P:(e + 1) * P, qsl_l], in_=y2l[:])

    nc.compile()
    return nc


def make_host_masks():
    import ml_dtypes

    m = np.zeros((len(MASK_DELTAS), P, QC), dtype=np.float32)
    ki = np.arange(P)[:, None]
    qi = np.arange(QC)[None, :]
    for i, d in enumerate(MASK_DELTAS):
        dist = d + qi - ki
        m[i] = ((dist >= 0) & (dist < WINDOW)).astype(np.float32)
    return m.astype(ml_dtypes.bfloat16)


def make_in_maps(x, cos, sin, Wq, Wk, Wv, Wo):
    import ml_dtypes

    bf = ml_dtypes.bfloat16
    cosT = np.ascontiguousarray(cos[:, 0, :].T)  # [128, S]
    sinT = sin[:, 0, :].T
    sinFT = np.concatenate([-sinT[: HD // 2], sinT[HD // 2:]], axis=0)
    sinFT = np.ascontiguousarray(sinFT.astype(np.float32))
    masks = make_host_masks()
    in_maps = []
    for c in range(8):
        b, g = c // 4, c % 4
        wq_g = Wq[g * 4 * HD:(g + 1) * 4 * HD, :]  # [512, E]
        wk_g = Wk[g * HD:(g + 1) * HD, :]  # [128, E]
        wv_g = Wv[g * HD:(g + 1) * HD, :]
        wqkvT = np.ascontiguousarray(
            np.concatenate([wq_g, wk_g, wv_g], axis=0).T
        ).astype(bf)  # [E, 768]
        woT_g = np.ascontiguousarray(
            Wo[:, g * 4 * HD:(g + 1) * 4 * HD].T
        ).astype(bf)  # [512, E]
        in_maps.append(
            {
                "xT": np.ascontiguousarray(x[b].T).astype(bf),
                "wqkvT": wqkvT,
                "woT": woT_g,
                "cosT": cosT,
                "sinFT": sinFT,
                "masks": masks,
            }
        )
    return in_maps


_NC_CACHE = {}


def get_nc():
    if "nc" not in _NC_CACHE:
        _NC_CACHE["nc"] = build_nc()
    return _NC_CACHE["nc"]


def kernel(x, cos, sin, Wq, Wk, Wv, Wo):
    from concourse.bass_utils import run_bass_kernel_spmd

    x = np.asarray(x, dtype=np.float32)
    cos = np.asarray(cos, dtype=np.float32)
    sin = np.asarray(sin, dtype=np.float32)
    Wq = np.asarray(Wq, dtype=np.float32)
    Wk = np.asarray(Wk, dtype=np.float32)
    Wv = np.asarray(Wv, dtype=np.float32)
    Wo = np.asarray(Wo, dtype=np.float32)

    nc = get_nc()
    in_maps = make_in_maps(x, cos, sin, Wq, Wk, Wv, Wo)
    res = run_bass_kernel_spmd(nc, in_maps, core_ids=list(range(8)))
    out = np.zeros((B, S, E), dtype=np.float32)
    for c in range(8):
        b = c // 4
        out[b] += res.results[c]["y"].T
    return out
